# revision 14
# baseline (speedup 1.0000x reference)
"""VQ codebook (CodebookEMA forward) Trainium2 kernel.

Full inputs -> shard batch axis over 8 NeuronCores (2 images/core) ->
Bass/Tile kernel per core -> gather/assemble full outputs.

Per-core pipeline (per batch image, [256, 4096] d-major token matrix):
  1. PE: scores x.w accumulated in PSUM [128 tok, 1024 codes]
     (codebook transposed on-chip once via PE transposes). Token tiles
     run [128 tokens x 1024 codes] per PSUM tile.
  2. DVE custom scan op: single-pass argmax_k of (x.w - |w|^2/2) read
     straight out of PSUM (bias row broadcast by partition_all_reduce).
  3. GPSIMD indirect DMA: gather codebook rows by token index from DRAM
     ([tok, d] tiles), then PE-transpose to zqT [c, tok] NCHW layout.
  4. DVE custom reduce: commitment loss partials sum((zq - x)^2).
Host: tiny assembly (idx de-permute, loss scalar, perplexity bincount).
"""
import sys

sys.path.insert(0, "/opt/trn_rl_repo")

import numpy as np
from operator import add

import concourse.bass as bass
import concourse.mybir as mybir
import concourse.tile as tile
import concourse.bass_isa as bass_isa
from concourse import bacc
from concourse.bass_utils import run_bass_kernel_spmd
from concourse.masks import make_identity

# problem constants (hardcoded per contract)
B, C, HH, WW = 16, 256, 64, 64
K = 1024
NCORES = 8
BPC = B // NCORES          # batches per core
TOK = HH * WW              # tokens per batch image
BETA = 0.25
F32 = mybir.dt.float32
F32R = mybir.dt.float32r
F16 = mybir.dt.float16
U16 = mybir.dt.uint16
I32 = mybir.dt.int32
I16 = mybir.dt.int16

NTT = TOK // 128           # token tiles per batch (32)
NR = 8                     # partition sub-groups (r) per tile
NQ = 16                    # wrapped group width (q)
GCH = 4                    # token tiles per indirect-gather chunk

# ---------------------------------------------------------------- custom ops
_OPS = {}


def _register_ops():
    if _OPS:
        return _OPS
    import concourse.dve_ops as dve_ops
    from concourse.dve_ops import DveOp
    from concourse.dve_spec import (
        Spec, Src0, Src1, C0, Idx, AluOp, MaxNeg, eq, select, maxx, sq, Scan,
        lower, _has_src1,
    )
    from concourse.dve_uop import DveOpSpec

    def make_op(name, spec, subdim=False):
        existing = {o.name: o for o in dve_ops.OPS}
        if name in existing:
            return existing[name]
        opcode = dve_ops._CUSTOM_DVE_ROW_BASE + len(dve_ops.OPS)
        shas = {}
        for ver in ("v3", "v4"):
            shas[ver] = DveOpSpec(
                name=name, opcode=opcode, uops=lower(spec, ver=ver),
                rd1_en=_has_src1(spec),
            ).sha(ver)
        op = DveOp(name, spec, subdim=subdim, uops_sha=shas)
        dve_ops.OPS.append(op)
        dve_ops.CUSTOM_DVE_SPECS[name] = spec
        dve_ops._SUB_OPCODE_FOR_NAME[name] = opcode
        return op

    def _ref_argmax_scan(in0, in1, s0, s1, imm2):
        b = (in0.astype(np.float32) - in1.astype(np.float32)).astype(np.float32)
        r = np.maximum.accumulate(b, axis=-1)
        n = b.shape[-1]
        idxs = np.arange(n, dtype=np.float32)
        marked = np.where(b == r, idxs, -np.finfo(np.float32).max)
        return marked, marked.reshape(marked.shape[0], -1).max(axis=-1, keepdims=True)

    def _ref_subsq_reduce(in0, in1, s0, s1, imm2):
        b = (in0.astype(np.float32) - in1.astype(np.float32)) ** 2
        return b.astype(np.float32), (
            s0 + b.reshape(b.shape[0], -1).sum(axis=-1, keepdims=True)
        )

    _b = Src0 - Src1
    _r = Scan(AluOp.MAX, _b)
    _OPS["argmax"] = make_op(
        "ARGMAX_SCAN_VQ",
        Spec(body=select(eq(_b, _r), Idx, MaxNeg), accum=maxx,
             reference=_ref_argmax_scan),
    )
    _OPS["subsq"] = make_op(
        "SUBSQ_REDUCE_VQ",
        Spec(body=sq(Src0 - Src1), accum=add, accum_init=C0,
             reference=_ref_subsq_reduce),
    )
    return _OPS


# ---------------------------------------------------------------- kernel build
_NC_CACHE = {}


def _build():
    if "nc" in _NC_CACHE:
        return _NC_CACHE["nc"]
    ops = _register_ops()
    nc = bacc.Bacc("TRN2", target_bir_lowering=False, debug=False,
                   num_devices=NCORES)

    x_in = nc.declare_dram_parameter("x", [BPC, C, TOK], F32, isOutput=False)
    cb_in = nc.declare_dram_parameter("cb", [K, C], F32, isOutput=False)
    wt_in = nc.declare_dram_parameter("wtT", [C, K], F32, isOutput=False)
    wb_in = nc.declare_dram_parameter("wbias", [1, K], F32, isOutput=False)
    z_out = nc.declare_dram_parameter("z", [BPC, C, TOK], F32, isOutput=True)
    idx_out = nc.declare_dram_parameter("idx", [BPC, TOK], I32, isOutput=True)
    stats_out = nc.declare_dram_parameter("stats", [128, 1], F32, isOutput=True)

    with tile.TileContext(nc) as tc:
        with tc.tile_pool(name="sbuf", bufs=1) as pool, \
             tc.tile_pool(name="psum", bufs=1, space="PSUM") as psum:
            # ---------------- setup: identity, codebook transpose, bias row
            ident = pool.tile([128, 128], F32, name="ident")
            make_identity(nc, ident[:])

            wh = [pool.tile([128, K], F16, name=f"wh{dt}") for dt in range(2)]
            wl = [pool.tile([128, K], F16, name=f"wl{dt}") for dt in range(2)]
            for dt in range(2):
                wtmp = pool.tile([128, K], F32, tag="gbuf",
                                 name=f"wtmp{dt}", bufs=4)
                nc.sync.dma_start(out=wtmp[:],
                                  in_=wt_in[dt * 128:(dt + 1) * 128, :])
                nc.scalar.activation(wh[dt][:], wtmp[:],
                                     mybir.ActivationFunctionType.Copy)
                nc.vector.tensor_tensor(wl[dt][:], wtmp[:], wh[dt][:],
                                        op=mybir.AluOpType.subtract)
            bias = pool.tile([128, K], F32, name="bias")
            nc.sync.dma_start(out=bias[0:1, :], in_=wb_in[:])
            nc.gpsimd.partition_broadcast(bias[:], bias[0:1, :], channels=128)

            stats_ssq = pool.tile([128, 2 * BPC * (NTT // GCH)], F32,
                                  name="stats_ssq")

            xb_all, gbuf_all = [], {}

            def phase2(b):
                zq = [pool.tile([128, TOK], F32, tag=f"zq{ct}",
                                name=f"zq{ct}_{b}", bufs=1) for ct in range(2)]
                nchunk = NTT // GCH
                for cc in range(nchunk):
                    gb = gbuf_all[(b, cc)]
                    g3 = gb[:].rearrange("p (t d) -> p t d", d=C)
                    cs = slice(cc * GCH * 128, (cc + 1) * GCH * 128)
                    for tt in range(cc * GCH, (cc + 1) * GCH):
                        for ct in range(2):
                            pz = psum.tile([128, 128], F32, tag="pst",
                                           name=f"pz_{b}_{tt}_{ct}", bufs=2)
                            nc.tensor.transpose(
                                pz[:], g3[:, tt % GCH,
                                          ct * 128:(ct + 1) * 128],
                                ident[:])
                            nc.scalar.activation(
                                zq[ct][:, tt * 128:(tt + 1) * 128], pz[:],
                                mybir.ActivationFunctionType.Copy)
                    for ct in range(2):
                        col = (2 * b + ct) * nchunk + cc
                        nc.sync.dma_start(
                            out=z_out[b, ct * 128:(ct + 1) * 128, cs],
                            in_=zq[ct][:, cs])
                        nc.vector._custom_dve(
                            ops["subsq"], out=zq[ct][:, cs],
                            in0=zq[ct][:, cs],
                            in1=xb_all[b][ct][:, cs], s0=0.0,
                            accum_out=stats_ssq[:, col:col + 1])

            # ---------------- phase 1 per batch: score + argmax + idx plumbing
            for b in range(BPC):
                xb = [pool.tile([128, TOK], F32, tag=f"xb{dt}",
                                name=f"xb{dt}_{b}", bufs=2) for dt in range(2)]
                xb_all.append(xb)
                xh = [pool.tile([128, TOK], F16, tag=f"xh{dt}",
                                name=f"xh{dt}_{b}", bufs=2) for dt in range(2)]
                xl = [pool.tile([128, TOK], F16, tag=f"xl{dt}",
                                name=f"xl{dt}_{b}", bufs=2) for dt in range(2)]
                NXQ = 8
                for q in range(NXQ):
                    qs = slice(q * (TOK // NXQ), (q + 1) * (TOK // NXQ))
                    for dt in range(2):
                        nc.sync.dma_start(
                            out=xb[dt][:, qs],
                            in_=x_in[b, dt * 128:(dt + 1) * 128, qs])
                        nc.scalar.activation(
                            xh[dt][:, qs], xb[dt][:, qs],
                            mybir.ActivationFunctionType.Copy)
                        nc.vector.tensor_tensor(
                            xl[dt][:, qs], xb[dt][:, qs], xh[dt][:, qs],
                            op=mybir.AluOpType.subtract)

                stats_idx = pool.tile([128, NTT], F32, tag="sidx",
                                      name=f"sidx_{b}", bufs=2)
                stats_i32 = pool.tile([128, NTT], I32, tag="sidx32",
                                      name=f"sidx32_{b}", bufs=2)
                for tt in range(NTT):
                    ps = psum.tile([128, K], F32, tag="ps", name=f"ps_{b}_{tt}",
                                   bufs=2)
                    ts = slice(tt * 128, (tt + 1) * 128)
                    # fp16 hi/lo split: x.w = xh.wh + xh.wl + xl.wh,
                    # grouped by stationary operand (4 LDWEIGHTS / tile)
                    plan = []
                    for dt in range(2):
                        for kc in range(2):
                            plan.append((xh[dt], wh[dt], kc))
                            plan.append((xh[dt], wl[dt], kc))
                    for dt in range(2):
                        for kc in range(2):
                            plan.append((xl[dt], wh[dt], kc))
                    cnt = {0: 0, 1: 0}
                    for _, _, kc in plan:
                        cnt[kc] += 1
                    seen = {0: 0, 1: 0}
                    for lhs, rhs, kc in plan:
                        seen[kc] += 1
                        nc.tensor.matmul(
                            ps[:, kc * 512:(kc + 1) * 512],
                            lhs[:, ts],
                            rhs[:, kc * 512:(kc + 1) * 512],
                            start=(seen[kc] == 1), stop=(seen[kc] == cnt[kc]),
                        )
                    nc.vector._custom_dve(
                        ops["argmax"], out=ps[:], in0=ps[:], in1=bias[:],
                        accum_out=stats_idx[:, tt:tt + 1])
                    if tt % GCH == GCH - 1:
                        cc = tt // GCH
                        sl = slice(cc * GCH, (cc + 1) * GCH)
                        nc.vector.tensor_copy(stats_i32[:, sl],
                                              stats_idx[:, sl])
                        gb = pool.tile([128, GCH * C], F32, tag="gbuf",
                                       name=f"gb_{b}_{cc}", bufs=4)
                        gbuf_all[(b, cc)] = gb
                        g3 = gb[:].rearrange("p (t d) -> p t d", d=C)
                        for j in range(GCH):
                            nc.gpsimd.indirect_dma_start(
                                out=g3[:, j],
                                out_offset=None,
                                in_=cb_in[:],
                                in_offset=bass.IndirectOffsetOnAxis(
                                    ap=stats_i32[:, cc * GCH + j:
                                                 cc * GCH + j + 1], axis=0),
                            )
                phase2(b)

                # idx -> token-major: psi[t, p] = idx(token 128t + p)
                psi = psum.tile([NTT, 128], F32, tag="psi", name=f"psi_{b}",
                                bufs=1)
                nc.tensor.transpose(psi[:], stats_idx[:], ident[:])
                idxT32 = pool.tile([NTT, 128], I32, tag="idxT32",
                                   name=f"idxT32_{b}", bufs=2)
                nc.vector.tensor_copy(idxT32[:], psi[:])
                nc.sync.dma_start(
                    out=idx_out[b].rearrange("(t p) -> t p", p=128),
                    in_=idxT32[:])


            stats_red = pool.tile([128, 1], F32, name="stats_red")
            nc.vector.reduce_sum(stats_red[:], stats_ssq[:],
                                 axis=mybir.AxisListType.X)
            nc.sync.dma_start(out=stats_out[:], in_=stats_red[:])

    nc.compile()
    _NC_CACHE["nc"] = nc
    return nc


# ---------------------------------------------------------------- entry point
def kernel(inputs: np.ndarray, codebook: np.ndarray):
    inputs = np.ascontiguousarray(np.asarray(inputs, dtype=np.float32))
    codebook = np.ascontiguousarray(np.asarray(codebook, dtype=np.float32))
    nc = _build()

    x_flat = inputs.reshape(B, C, TOK)
    wtT = np.ascontiguousarray(codebook.T)
    wbias = np.ascontiguousarray(
        (0.5 * (codebook.astype(np.float64) ** 2).sum(axis=1))
        .astype(np.float32)[None, :])
    in_maps = [
        {"x": np.ascontiguousarray(x_flat[c * BPC:(c + 1) * BPC]),
         "cb": codebook, "wtT": wtT, "wbias": wbias}
        for c in range(NCORES)
    ]
    _r = run_bass_kernel_spmd(nc, in_maps, list(range(NCORES)))
    globals()["LAST_RESULTS"] = _r
    res = _r.results

    z = np.concatenate([r["z"] for r in res], axis=0).reshape(B, C, HH, WW)
    idx = np.concatenate([r["idx"].reshape(-1) for r in res]).astype(np.int32)
    ssq = float(sum(r["stats"].astype(np.float64).sum() for r in res))

    n_tokens = B * TOK
    loss = np.float32(BETA * ssq / (n_tokens * C))
    counts = np.bincount(idx, minlength=K).astype(np.float64)
    avg = counts / n_tokens
    perplexity = np.float32(np.exp(-np.sum(avg * np.log(avg + 1e-10))))
    return z, loss, perplexity, idx[:, None].astype(np.int32)


# revision 15
# speedup vs baseline: 1.0248x; 1.0248x over previous
"""VQ codebook (CodebookEMA forward) Trainium2 kernel.

Full inputs -> shard batch axis over 8 NeuronCores (2 images/core) ->
Bass/Tile kernel per core -> gather/assemble full outputs.

Per-core pipeline (per batch image, [256, 4096] d-major token matrix):
  1. PE: scores x.w accumulated in PSUM [128 tok, 1024 codes]
     (codebook transposed on-chip once via PE transposes). Token tiles
     run [128 tokens x 1024 codes] per PSUM tile.
  2. DVE custom scan op: single-pass argmax_k of (x.w - |w|^2/2) read
     straight out of PSUM (bias row broadcast by partition_all_reduce).
  3. GPSIMD indirect DMA: gather codebook rows by token index from DRAM
     ([tok, d] tiles), then PE-transpose to zqT [c, tok] NCHW layout.
  4. DVE custom reduce: commitment loss partials sum((zq - x)^2).
Host: tiny assembly (idx de-permute, loss scalar, perplexity bincount).
"""
import sys

sys.path.insert(0, "/opt/trn_rl_repo")

import numpy as np
from operator import add

import concourse.bass as bass
import concourse.mybir as mybir
import concourse.tile as tile
import concourse.bass_isa as bass_isa
from concourse import bacc
from concourse.bass_utils import run_bass_kernel_spmd
from concourse.masks import make_identity

# problem constants (hardcoded per contract)
B, C, HH, WW = 16, 256, 64, 64
K = 1024
NCORES = 8
BPC = B // NCORES          # batches per core
TOK = HH * WW              # tokens per batch image
BETA = 0.25
F32 = mybir.dt.float32
F32R = mybir.dt.float32r
F16 = mybir.dt.float16
U16 = mybir.dt.uint16
I32 = mybir.dt.int32
I16 = mybir.dt.int16

NTT = TOK // 128           # token tiles per batch (32)
NR = 8                     # partition sub-groups (r) per tile
NQ = 16                    # wrapped group width (q)
GCH = 8                    # token tiles per indirect-gather chunk

# ---------------------------------------------------------------- custom ops
_OPS = {}


def _register_ops():
    if _OPS:
        return _OPS
    import concourse.dve_ops as dve_ops
    from concourse.dve_ops import DveOp
    from concourse.dve_spec import (
        Spec, Src0, Src1, C0, Idx, AluOp, MaxNeg, eq, select, maxx, sq, Scan,
        lower, _has_src1,
    )
    from concourse.dve_uop import DveOpSpec

    def make_op(name, spec, subdim=False):
        existing = {o.name: o for o in dve_ops.OPS}
        if name in existing:
            return existing[name]
        opcode = dve_ops._CUSTOM_DVE_ROW_BASE + len(dve_ops.OPS)
        shas = {}
        for ver in ("v3", "v4"):
            shas[ver] = DveOpSpec(
                name=name, opcode=opcode, uops=lower(spec, ver=ver),
                rd1_en=_has_src1(spec),
            ).sha(ver)
        op = DveOp(name, spec, subdim=subdim, uops_sha=shas)
        dve_ops.OPS.append(op)
        dve_ops.CUSTOM_DVE_SPECS[name] = spec
        dve_ops._SUB_OPCODE_FOR_NAME[name] = opcode
        return op

    def _ref_argmax_scan(in0, in1, s0, s1, imm2):
        b = (in0.astype(np.float32) - in1.astype(np.float32)).astype(np.float32)
        r = np.maximum.accumulate(b, axis=-1)
        n = b.shape[-1]
        idxs = np.arange(n, dtype=np.float32)
        marked = np.where(b == r, idxs, -np.finfo(np.float32).max)
        return marked, marked.reshape(marked.shape[0], -1).max(axis=-1, keepdims=True)

    def _ref_subsq_reduce(in0, in1, s0, s1, imm2):
        b = (in0.astype(np.float32) - in1.astype(np.float32)) ** 2
        return b.astype(np.float32), (
            s0 + b.reshape(b.shape[0], -1).sum(axis=-1, keepdims=True)
        )

    _b = Src0 - Src1
    _r = Scan(AluOp.MAX, _b)
    _OPS["argmax"] = make_op(
        "ARGMAX_SCAN_VQ",
        Spec(body=select(eq(_b, _r), Idx, MaxNeg), accum=maxx,
             reference=_ref_argmax_scan),
    )
    _OPS["subsq"] = make_op(
        "SUBSQ_REDUCE_VQ",
        Spec(body=sq(Src0 - Src1), accum=add, accum_init=C0,
             reference=_ref_subsq_reduce),
    )
    return _OPS


# ---------------------------------------------------------------- kernel build
_NC_CACHE = {}


def _build():
    if "nc" in _NC_CACHE:
        return _NC_CACHE["nc"]
    ops = _register_ops()
    nc = bacc.Bacc("TRN2", target_bir_lowering=False, debug=False,
                   num_devices=NCORES)

    x_in = nc.declare_dram_parameter("x", [BPC, C, TOK], F32, isOutput=False)
    cb_in = nc.declare_dram_parameter("cb", [K, C], F32, isOutput=False)
    wt_in = nc.declare_dram_parameter("wtT", [C, K], F32, isOutput=False)
    wb_in = nc.declare_dram_parameter("wbias", [1, K], F32, isOutput=False)
    z_out = nc.declare_dram_parameter("z", [BPC, C, TOK], F32, isOutput=True)
    idx_out = nc.declare_dram_parameter("idx", [BPC, TOK], I32, isOutput=True)
    stats_out = nc.declare_dram_parameter("stats", [128, 1], F32, isOutput=True)

    with tile.TileContext(nc) as tc:
        with tc.tile_pool(name="sbuf", bufs=1) as pool, \
             tc.tile_pool(name="psum", bufs=1, space="PSUM") as psum:
            # ---------------- setup: identity, codebook transpose, bias row
            ident = pool.tile([128, 128], F32, name="ident")
            make_identity(nc, ident[:])

            wh = [pool.tile([128, K], F16, name=f"wh{dt}") for dt in range(2)]
            wl = [pool.tile([128, K], F16, name=f"wl{dt}") for dt in range(2)]
            for dt in range(2):
                wtmp = pool.tile([128, K], F32, tag="gbuf",
                                 name=f"wtmp{dt}", bufs=4)
                nc.sync.dma_start(out=wtmp[:],
                                  in_=wt_in[dt * 128:(dt + 1) * 128, :])
                nc.scalar.activation(wh[dt][:], wtmp[:],
                                     mybir.ActivationFunctionType.Copy)
                nc.vector.tensor_tensor(wl[dt][:], wtmp[:], wh[dt][:],
                                        op=mybir.AluOpType.subtract)
            bias = pool.tile([128, K], F32, name="bias")
            nc.sync.dma_start(out=bias[0:1, :], in_=wb_in[:])
            nc.gpsimd.partition_broadcast(bias[:], bias[0:1, :], channels=128)

            stats_ssq = pool.tile([128, 2 * BPC * (NTT // GCH)], F32,
                                  name="stats_ssq")

            xb_all, gbuf_all = [], {}

            def phase2(b):
                zq = [pool.tile([128, TOK], F32, tag=f"zq{ct}",
                                name=f"zq{ct}_{b}", bufs=1) for ct in range(2)]
                nchunk = NTT // GCH
                for cc in range(nchunk):
                    gb = gbuf_all[(b, cc)]
                    g3 = gb[:].rearrange("p (t d) -> p t d", d=C)
                    cs = slice(cc * GCH * 128, (cc + 1) * GCH * 128)
                    for tt in range(cc * GCH, (cc + 1) * GCH):
                        for ct in range(2):
                            pz = psum.tile([128, 128], F32, tag="pst",
                                           name=f"pz_{b}_{tt}_{ct}", bufs=2)
                            nc.tensor.transpose(
                                pz[:], g3[:, tt % GCH,
                                          ct * 128:(ct + 1) * 128],
                                ident[:])
                            nc.scalar.activation(
                                zq[ct][:, tt * 128:(tt + 1) * 128], pz[:],
                                mybir.ActivationFunctionType.Copy)
                    for ct in range(2):
                        col = (2 * b + ct) * nchunk + cc
                        nc.sync.dma_start(
                            out=z_out[b, ct * 128:(ct + 1) * 128, cs],
                            in_=zq[ct][:, cs])
                        nc.vector._custom_dve(
                            ops["subsq"], out=zq[ct][:, cs],
                            in0=zq[ct][:, cs],
                            in1=xb_all[b][ct][:, cs], s0=0.0,
                            accum_out=stats_ssq[:, col:col + 1])

            # ---------------- phase 1 per batch: score + argmax + idx plumbing
            for b in range(BPC):
                xb = [pool.tile([128, TOK], F32, tag=f"xb{dt}",
                                name=f"xb{dt}_{b}", bufs=2) for dt in range(2)]
                xb_all.append(xb)
                xh = [pool.tile([128, TOK], F16, tag=f"xh{dt}",
                                name=f"xh{dt}_{b}", bufs=2) for dt in range(2)]
                xl = [pool.tile([128, TOK], F16, tag=f"xl{dt}",
                                name=f"xl{dt}_{b}", bufs=2) for dt in range(2)]
                NXQ = 4
                for q in range(NXQ):
                    qs = slice(q * (TOK // NXQ), (q + 1) * (TOK // NXQ))
                    for dt in range(2):
                        nc.sync.dma_start(
                            out=xb[dt][:, qs],
                            in_=x_in[b, dt * 128:(dt + 1) * 128, qs])
                        nc.scalar.activation(
                            xh[dt][:, qs], xb[dt][:, qs],
                            mybir.ActivationFunctionType.Copy)
                        nc.vector.tensor_tensor(
                            xl[dt][:, qs], xb[dt][:, qs], xh[dt][:, qs],
                            op=mybir.AluOpType.subtract)

                stats_idx = pool.tile([128, NTT], F32, tag="sidx",
                                      name=f"sidx_{b}", bufs=2)
                stats_i32 = pool.tile([128, NTT], I32, tag="sidx32",
                                      name=f"sidx32_{b}", bufs=2)
                for tt in range(NTT):
                    ps = psum.tile([128, K], F32, tag="ps", name=f"ps_{b}_{tt}",
                                   bufs=2)
                    ts = slice(tt * 128, (tt + 1) * 128)
                    # fp16 hi/lo split: x.w = xh.wh + xh.wl + xl.wh,
                    # grouped by stationary operand (4 LDWEIGHTS / tile)
                    plan = []
                    for dt in range(2):
                        for kc in range(2):
                            plan.append((xh[dt], wh[dt], kc))
                            plan.append((xh[dt], wl[dt], kc))
                    for dt in range(2):
                        for kc in range(2):
                            plan.append((xl[dt], wh[dt], kc))
                    cnt = {0: 0, 1: 0}
                    for _, _, kc in plan:
                        cnt[kc] += 1
                    seen = {0: 0, 1: 0}
                    for lhs, rhs, kc in plan:
                        seen[kc] += 1
                        nc.tensor.matmul(
                            ps[:, kc * 512:(kc + 1) * 512],
                            lhs[:, ts],
                            rhs[:, kc * 512:(kc + 1) * 512],
                            start=(seen[kc] == 1), stop=(seen[kc] == cnt[kc]),
                        )
                    nc.vector._custom_dve(
                        ops["argmax"], out=ps[:], in0=ps[:], in1=bias[:],
                        accum_out=stats_idx[:, tt:tt + 1])
                    if tt % GCH == GCH - 1:
                        cc = tt // GCH
                        sl = slice(cc * GCH, (cc + 1) * GCH)
                        nc.vector.tensor_copy(stats_i32[:, sl],
                                              stats_idx[:, sl])
                        gb = pool.tile([128, GCH * C], F32, tag="gbuf",
                                       name=f"gb_{b}_{cc}", bufs=4)
                        gbuf_all[(b, cc)] = gb
                        g3 = gb[:].rearrange("p (t d) -> p t d", d=C)
                        for j in range(GCH):
                            nc.gpsimd.indirect_dma_start(
                                out=g3[:, j],
                                out_offset=None,
                                in_=cb_in[:],
                                in_offset=bass.IndirectOffsetOnAxis(
                                    ap=stats_i32[:, cc * GCH + j:
                                                 cc * GCH + j + 1], axis=0),
                            )
                phase2(b)

                # idx -> token-major: psi[t, p] = idx(token 128t + p)
                psi = psum.tile([NTT, 128], F32, tag="psi", name=f"psi_{b}",
                                bufs=1)
                nc.tensor.transpose(psi[:], stats_idx[:], ident[:])
                idxT32 = pool.tile([NTT, 128], I32, tag="idxT32",
                                   name=f"idxT32_{b}", bufs=2)
                nc.vector.tensor_copy(idxT32[:], psi[:])
                nc.sync.dma_start(
                    out=idx_out[b].rearrange("(t p) -> t p", p=128),
                    in_=idxT32[:])


            stats_red = pool.tile([128, 1], F32, name="stats_red")
            nc.vector.reduce_sum(stats_red[:], stats_ssq[:],
                                 axis=mybir.AxisListType.X)
            nc.sync.dma_start(out=stats_out[:], in_=stats_red[:])

    nc.compile()
    _NC_CACHE["nc"] = nc
    return nc


# ---------------------------------------------------------------- entry point
def kernel(inputs: np.ndarray, codebook: np.ndarray):
    inputs = np.ascontiguousarray(np.asarray(inputs, dtype=np.float32))
    codebook = np.ascontiguousarray(np.asarray(codebook, dtype=np.float32))
    nc = _build()

    x_flat = inputs.reshape(B, C, TOK)
    wtT = np.ascontiguousarray(codebook.T)
    wbias = np.ascontiguousarray(
        (0.5 * (codebook.astype(np.float64) ** 2).sum(axis=1))
        .astype(np.float32)[None, :])
    in_maps = [
        {"x": np.ascontiguousarray(x_flat[c * BPC:(c + 1) * BPC]),
         "cb": codebook, "wtT": wtT, "wbias": wbias}
        for c in range(NCORES)
    ]
    _r = run_bass_kernel_spmd(nc, in_maps, list(range(NCORES)))
    globals()["LAST_RESULTS"] = _r
    res = _r.results

    z = np.concatenate([r["z"] for r in res], axis=0).reshape(B, C, HH, WW)
    idx = np.concatenate([r["idx"].reshape(-1) for r in res]).astype(np.int32)
    ssq = float(sum(r["stats"].astype(np.float64).sum() for r in res))

    n_tokens = B * TOK
    loss = np.float32(BETA * ssq / (n_tokens * C))
    counts = np.bincount(idx, minlength=K).astype(np.float64)
    avg = counts / n_tokens
    perplexity = np.float32(np.exp(-np.sum(avg * np.log(avg + 1e-10))))
    return z, loss, perplexity, idx[:, None].astype(np.int32)
